# revision 42
# baseline (speedup 1.0000x reference)
"""Multi-head causal attention (B=2, S=2048, E=1024, H=16, D=64) on 8 TRN2 cores.

Sharding: core c -> batch b = c // 4, head group g = c % 4 (4 heads each).
Each core computes q/k/v projections + RoPE + causal attention + its rows of
the Wo projection for its (batch, head-group); the host sums the 4 row-parallel
Wo partials per batch.

v3 design (vs v2):
  - Z-replication: v tiles are [v(64) | ones(64)] per head, so the AV matmul's
    otherwise-idle output partitions 64:128 produce the softmax denominator
    replicated across 64 partitions for free.  Normalization is then one
    reciprocal + two muls straight out of psum (no drain copy, no
    partition_broadcast, no gpsimd muls).
  - Causal mask multiply runs on the Pool engine (DVE was near-saturated).
  - RoPE: one copy drains the projection psum to f16 SBUF (ACT for the
    first 8 s-blocks, where ACT is idle and the seam's DVE queue gates the
    psum rotation; DVE later), then the rope muls run in DVE 2x mode.
  - AV runs LAG=2 steps behind scores, so exp/mask have 2 steps of slack
    and each pair's first AV lands past the previous pair's norm WAR.
  - The final out-projection pair is carried into the next body and the next
    body's weight/x loads are prefetched, so the PE never idles across the
    body seam (the PE p-state ramp makes every idle gap cost ~2x for 3us).
  - wqk is double-buffered by body parity; rope/mask tables load once.
  - Projection / v-projection / out-projection work is sliced into ~426ns
    granules fed as fillers into every attention step (one per step), with
    deadlines chosen so transposes/v-chunks are ready exactly when needed.

Measured (differential u9-u1 marginal, 8-core axon trn2): 187.5us vs the
v2 baseline's 213-229us.  TimelineSim (cost model) marginal: 134.1us.
Model-better variants (DVE masks, LAG=3, finer carry interleave; model
124.8us) measured WORSE on hardware (199-216us) - the model undercounts
DVE-chain and sem effects, so changes here should be re-validated on HW.
"""

import sys

if "/opt/trn_rl_repo" not in sys.path:
    sys.path.insert(0, "/opt/trn_rl_repo")

import numpy as np

import concourse.bass as bass
import concourse.tile as tile
from concourse import bacc, mybir
from concourse.bass_utils import run_bass_kernel_spmd

B, S, E, H, D = 2, 2048, 1024, 16, 64
HPC = 4  # heads per core
NCORES = 8
SB = 512  # q block width (attention)
NSB = S // SB  # 4
KT = 128  # k tile (partition chunk of the sequence)
PB = 128  # projection s-block
NPB = S // PB  # 16
ECH = E // 128  # 8 contraction chunks for the projections

f32 = mybir.dt.float32
f16 = mybir.dt.float16

ROPE_BASE = 10000.0


def build_nc(unroll=1):
    nc = bacc.Bacc(
        "TRN2", target_bir_lowering=False, debug=False, enable_asserts=False
    )

    xT_d = nc.dram_tensor("xT", [E, S], f16, kind="ExternalInput")
    wqk_d = nc.dram_tensor("wqk", [E, 2 * HPC * D], f16, kind="ExternalInput")
    wv_d = nc.dram_tensor("wv", [E, HPC * D], f16, kind="ExternalInput")
    wo_d = nc.dram_tensor("wo", [HPC * D, E], f16, kind="ExternalInput")
    cos_d = nc.dram_tensor("cosT", [128, NPB, 32], f16, kind="ExternalInput")
    sin_d = nc.dram_tensor("sinT", [128, NPB, 64], f16, kind="ExternalInput")
    mask_d = nc.dram_tensor("maskc", [128, 128], f16, kind="ExternalInput")
    out_d = nc.dram_tensor("out", [S, E], f16, kind="ExternalOutput")

    with tile.TileContext(nc) as tc:
        with (
            tc.tile_pool(name="const", bufs=1) as constp,
            tc.tile_pool(name="rope", bufs=2) as ropep,
            tc.tile_pool(name="st", bufs=4) as stp,
            tc.tile_pool(name="nrm", bufs=2) as nrmp,
            tc.tile_pool(name="mm", bufs=2, space="PSUM") as mmp,
            tc.tile_pool(name="sc", bufs=2, space="PSUM") as scp,
            tc.tile_pool(name="acc", bufs=1, space="PSUM") as accp,
        ):
            # ---- SBUF residents -------------------------------------------
            # x is split over two ping-pong tiles (a: q-blocks 0,2 / b: 1,3)
            # so a later 512-block load never false-serializes against the
            # readers of the other tile (dep tracking is tile-granular).
            xT_ap = xT_d.ap().rearrange("(eo p) s -> p eo s", p=128)
            xTt = [
                constp.tile([128, ECH, S // 2], f16, tag=f"xT{i}", name=f"xT{i}")
                for i in range(2)
            ]

            def xT(sb):
                # lhsT slice [128, 128] for projection s-block sb
                b = sb // 4
                return xTt[b % 2][
                    :, :, (b // 2) * SB + (sb % 4) * PB : (b // 2) * SB + (sb % 4 + 1) * PB
                ]

            # wqk double-buffered by body parity: the next body's reload never
            # waits on this body's late projection readers.
            wqk2 = [
                constp.tile([128, ECH, 2 * HPC * D], f16, tag=f"wqk{u}",
                            name=f"wqk{u}")
                for u in range(2)
            ]
            wqk_ap = wqk_d.ap().rearrange("(eo p) m -> p eo m", p=128)
            wv = constp.tile([128, ECH, HPC * D], f16, tag="wv")
            wo = constp.tile([128, 2, E], f16, tag="wo")
            cosT = constp.tile([128, NPB, 32], f16, tag="cosT")
            sinT = constp.tile([128, NPB, 64], f16, tag="sinT")
            maskc = constp.tile([128, 128], f16, tag="maskc")

            # qkT slots: 0,1 = q pairs (heads 01 / 23), 2,3 = k pairs.
            # Body-scoped tensors are double-buffered by unroll parity so the
            # next body's writes never serialize against this body's readers.
            qkbig2 = [
                constp.tile([128, 4, S], f16, tag=f"qkbig{u}", name=f"qkbig{u}")
                for u in range(2)
            ]
            # v per k-chunk: 4 heads x [v_h (64) | ones (64)].  The ones
            # half makes the AV matmul emit the softmax denominator
            # replicated across output partitions 64:128 (free Z-broadcast).
            v_big2 = [
                constp.tile(
                    [128, S // KT, HPC * 128], f16, tag=f"vbig{u}", name=f"vbig{u}"
                )
                for u in range(2)
            ]
            for u in range(2):
                ones_cols = v_big2[u].rearrange("p n (h m) -> p n h m", h=HPC)[
                    :, :, :, 64:128
                ]
                nc.vector.memset(ones_cols, 1.0)

            at8_2 = {}
            for u in range(2):
                for c in range(2):
                    for qb in range(NSB):
                        at8_2[(u, c, qb)] = constp.tile(
                            [128, SB], f16, tag=f"at{u}_{c}_{qb}",
                            name=f"at{u}_{c}_{qb}",
                        )

            # ---- DMA emission helpers -------------------------------------
            def emit_load_head(it):
                # consumption-ordered, finely interleaved per 2-chunk pair so
                # the first s-block's contraction never runs dry. wv goes on
                # the scalar-engine HWDGE queue (which later carries the qkT
                # XBAR transposes and wo).
                wqk = cur["wqk"]
                for e in range(0, ECH, 2):
                    nc.sync.dma_start(
                        out=wqk[:, e : e + 2, :], in_=wqk_ap[:, e : e + 2, :]
                    )
                    nc.sync.dma_start(
                        out=xTt[0][:, e : e + 2, 0:SB],
                        in_=xT_ap[:, e : e + 2, 0:SB],
                    )
                    if e == 2 and it == 0:
                        nc.sync.dma_start(out=cosT, in_=cos_d.ap())
                        nc.sync.dma_start(out=sinT, in_=sin_d.ap())
                    if e == 4:
                        nc.scalar.dma_start(out=wv, in_=wv_d.ap().rearrange(
                            "(eo p) m -> p eo m", p=128))
                if it == 0:
                    nc.sync.dma_start(out=maskc, in_=mask_d.ap())

            def emit_loads(b):
                cs = slice(b * SB, (b + 1) * SB)
                dst = slice((b // 2) * SB, (b // 2 + 1) * SB)
                nc.sync.dma_start(
                    out=xTt[b % 2][:, :, dst], in_=xT_ap[:, :, cs]
                )

            def emit_load_wo():
                nc.scalar.dma_start(
                    out=wo, in_=wo_d.ap().rearrange("(c p) e -> p c e", p=128)
                )

            # ---- per-s-block projection + rope + transpose ----------------
            pending_t = []
            cur = {}

            def flush_transposes(upto=99):
                while pending_t and pending_t[0][0] <= upto:
                    sb, rout = pending_t.pop(0)
                    # one fused XBAR transpose: [128 s, 4*128 f] ->
                    # qkbig[dd, slot, s] (per-slot 128x128 transpose)
                    nc.scalar.dma_start_transpose(
                        out=cur["qk"][:, :, sb * PB : (sb + 1) * PB],
                        in_=rout,
                    )

            def gen_qk_proj(sb):
                """Returns granules (2 contraction chunks each, ~426ns of PE)
                so the matmuls can be interleaved between attention steps as
                PE gap-filler; the last granule runs rope."""
                state = {}

                def chunk(e0):
                    def f():
                        if "ps" not in state:
                            state["ps"] = mmp.tile(
                                [128, 512], f32, tag="mm", name="ps"
                            )
                        xts = xT(sb)
                        wqk = cur["wqk"]
                        for e in (e0, e0 + 1):
                            nc.tensor.matmul(
                                out=state["ps"],
                                lhsT=xts[:, e, :],
                                rhs=wqk[:, e, :],
                                start=(e == 0),
                                stop=(e == ECH - 1),
                            )
                        if e0 == ECH - 2:
                            emit_rope(sb, state["ps"])
                    return f

                return [chunk(e) for e in range(0, ECH, 2)]

            def emit_qk_proj(sb):
                for f in gen_qk_proj(sb):
                    f()

            def emit_rope(sb, ps):
                # drain psum once (frees the mm buffer after a single read),
                # then rope runs on DVE in 2x mode (all-SBUF 16-bit operands).
                # Early s-blocks drain on ACT (idle before the exp stream
                # starts, and the seam's DVE queue is deep in norm+rope work);
                # later ones drain on DVE (ACT is saturated by exps there).
                rt = ropep.tile([128, 512], f16, tag="rt", name="rt")
                if sb < 8:
                    nc.scalar.copy(out=rt, in_=ps)
                else:
                    nc.vector.tensor_copy(out=rt, in_=ps)
                # rope: cols = [q(4h) | k(4h)], per head [x1(32) | x2(32)]
                t1 = ropep.tile([128, 512], f16, tag="t1", name="t1")
                t2 = ropep.tile([128, 512], f16, tag="t2", name="t2")
                rout = ropep.tile([128, 512], f16, tag="ro", name="ro", bufs=3)
                rt4 = rt.rearrange("p (g two i) -> p g two i", two=2, i=32)
                t24 = t2.rearrange("p (g two i) -> p g two i", two=2, i=32)
                eng = nc.vector
                eng.tensor_mul(
                    t1.rearrange("p (g i) -> p g i", i=32),
                    rt.rearrange("p (g i) -> p g i", i=32),
                    cosT[:, sb, None, :].broadcast_to((128, 16, 32)),
                )
                # o1 part: -x2*sin ; o2 part: +x1*sin
                eng.tensor_mul(
                    t24[:, :, 0:1, :],
                    rt4[:, :, 1:2, :],
                    sinT[:, sb, None, None, 0:32].broadcast_to((128, 8, 1, 32)),
                )
                eng.tensor_mul(
                    t24[:, :, 1:2, :],
                    rt4[:, :, 0:1, :],
                    sinT[:, sb, None, None, 32:64].broadcast_to((128, 8, 1, 32)),
                )
                eng.tensor_add(rout, t1, t2)
                # defer the XBAR transpose so its rope dependency is already
                # satisfied when the scalar queue reaches it (no head-of-line
                # blocking of the exp stream)
                pending_t.append((sb, rout))

            def gen_v_proj(sb):
                state = {}

                def chunk(e0):
                    def f():
                        if "pv" not in state:
                            state["pv"] = mmp.tile(
                                [128, 512], f32, tag="mm", name="pv"
                            )
                        xts = xT(sb)
                        for e in (e0, e0 + 1):
                            nc.tensor.matmul(
                                out=state["pv"][:, 0 : HPC * D],
                                lhsT=xts[:, e, :],
                                rhs=wv[:, e, :],
                                start=(e == 0),
                                stop=(e == ECH - 1),
                            )
                        if e0 == ECH - 2:
                            nc.vector.tensor_copy(
                                out=cur["v"].rearrange(
                                    "p n (h m) -> p n h m", h=HPC
                                )[:, sb, :, 0:64],
                                in_=state["pv"][:, 0 : HPC * D].rearrange(
                                    "p (h m) -> p h m", h=HPC
                                ),
                            )
                    return f

                return [chunk(e) for e in range(0, ECH, 2)]

            def emit_v_proj(sb):
                for f in gen_v_proj(sb):
                    f()

            # ---- attention ------------------------------------------------
            # AV runs LAG steps behind scores: the first AV of a pair lands
            # ~3 PE-ops after the pair boundary (covering the previous pair's
            # psum-WAR on the normalization reads), and exp/mask get 2 steps
            # of slack before their AV consumer.
            LAG = 2

            def emit_attn(qb, p, av2, fillers=()):
                fillers = list(fillers)
                n_k = 4 * (qb + 1)
                sts = {}
                for step in range(n_k + LAG):
                    # PE gap-filler: the exp latency leaves PE slack per
                    # step; spend it on proj/out-proj granules
                    if step > 0 and fillers:
                        fillers.pop(0)()
                    if step < n_k:
                        t = step
                        j = t - 4 * qb  # >= 0 on diagonal blocks
                        w = SB - KT * j if j >= 0 else SB
                        offs = KT * j if j >= 0 else 0
                        sct = scp.tile([128, 2 * SB], f32, tag="sc", name="sct")
                        for i in range(2):
                            hb = 64 * i
                            nc.tensor.matmul(
                                out=sct[:, i * SB : i * SB + w],
                                lhsT=cur["qk"][hb : hb + 64, 2 + p, t * KT : (t + 1) * KT],
                                rhs=cur["qk"][
                                    hb : hb + 64, p, qb * SB + offs : (qb + 1) * SB
                                ],
                                start=True,
                                stop=True,
                            )
                        st = stp.tile([128, 2 * SB], f16, tag="st", name="st", bufs=LAG + 2)
                        sc3 = sct.rearrange("p (i c) -> p i c", i=2)
                        st3 = st.rearrange("p (i c) -> p i c", i=2)
                        nc.scalar.activation(
                            out=st3[:, :, 0:w],
                            in_=sc3[:, :, 0:w],
                            func=mybir.ActivationFunctionType.Exp,
                            scale=0.125,
                        )
                        if j >= 0:
                            # causal mask on the diagonal 128-col block; Pool
                            # engine keeps it out of the DVE queue
                            nc.gpsimd.tensor_mul(
                                st3[:, :, 0:128],
                                st3[:, :, 0:128],
                                maskc[:, None, :].broadcast_to((128, 2, 128)),
                            )
                        sts[t] = (st, w, offs)
                    if step >= LAG:
                        t = step - LAG
                        st, w, offs = sts.pop(t)
                        for i in range(2):
                            h = 2 * p + i
                            nc.tensor.matmul(
                                out=av2[:, i * SB + offs : (i + 1) * SB],
                                lhsT=cur["v"][:, t, h * 128 : (h + 1) * 128],
                                rhs=st[:, i * SB : i * SB + w],
                                start=(t == 0),
                                stop=(t == n_k - 1),
                            )

            def emit_attn_pair(qb, p, fillers=()):
                av2 = accp.tile([128, 2 * SB], f32, tag="acc", name="av2")
                emit_attn(qb, p, av2, fillers)
                # av2 rows 64:128 hold Z replicated per head; normalize the
                # numerator rows straight out of psum.
                zinv = nrmp.tile([64, 2 * SB], f32, tag="zi", name="zi")
                nc.vector.reciprocal(out=zinv, in_=av2[64:128, :])
                for i in range(2):
                    nc.vector.tensor_mul(
                        cur["at"][(p, qb)][64 * i : 64 * i + 64, :],
                        av2[0:64, i * SB : (i + 1) * SB],
                        zinv[:, i * SB : (i + 1) * SB],
                    )

            # ---- output projection (row-parallel partial) -----------------
            out_ap = out_d.ap().rearrange(
                "(qb stl p) (eb c) -> qb stl p eb c", p=128, stl=4, c=512
            )

            def gen_out_proj(qb, eb, drain_act=False, use_sc=False):
                state = {}
                at = cur["at"]

                def group(stl):
                    def f():
                        if "ot" not in state:
                            state["ot"] = stp.tile(
                                [128, 4, 512], f16, tag="ot", name="ot", bufs=2
                            )
                        if use_sc:
                            # head-of-body carry: scores pool is idle there
                            pw = scp.tile(
                                [128, 2 * SB], f32, tag="sc", name="pws"
                            )[:, 0:512]
                        else:
                            pw = mmp.tile([128, 512], f32, tag="mm", name="pw")
                        for c in range(2):
                            nc.tensor.matmul(
                                out=pw,
                                lhsT=at[(c, qb)][
                                    :, stl * KT : (stl + 1) * KT
                                ],
                                rhs=wo[:, c, eb * 512 : (eb + 1) * 512],
                                start=(c == 0),
                                stop=(c == 1),
                            )
                        if drain_act:
                            # carried into the next body's head: ACT is idle
                            nc.scalar.copy(out=state["ot"][:, stl, :], in_=pw)
                        else:
                            nc.vector.tensor_copy(out=state["ot"][:, stl, :], in_=pw)
                        if stl == 3:
                            # scalar HWDGE queue: keeps the SP queue free for
                            # the next body's x/wqk input prefetch at the seam
                            nc.scalar.dma_start(
                                out=out_ap[qb, :, :, eb, :].rearrange(
                                    "stl p c -> p stl c"
                                ),
                                in_=state["ot"],
                            )
                    return f

                return [group(stl) for stl in range(4)]

            def emit_out_proj(qb, eb):
                for f in gen_out_proj(qb, eb):
                    f()

            # ---- emission schedule ---------------------------------------
            # Static software pipeline; one filler granule (~426ns of PE) per
            # attention step, deadline-safe:
            #   T(0..3) before attn(0,*); T(4..7) before attn(1,*); ...
            #   V(t) copy before the AV of chunk t; OP(qb,*) after norm(qb,1).
            # The final OP(3,1) is carried into the next body's head, where it
            # covers the norm(3,1) -> at(.,3) latency.
            carry = []
            for it in range(unroll):
                cur["qk"] = qkbig2[it % 2]
                cur["v"] = v_big2[it % 2]
                cur["wqk"] = wqk2[it % 2]
                cur["at"] = {
                    (c, qb): at8_2[(it % 2, c, qb)]
                    for c in range(2)
                    for qb in range(NSB)
                }
                emit_load_head(it)
                emit_qk_proj(0)
                # previous body's deferred final out-proj pair: its inputs
                # are ready here, so it fills PE time while this body's
                # first rope/transposes propagate (interleaved with the
                # early projections to smooth the mm-psum rotation)
                carry_a, carry_b = carry[: len(carry) // 2], carry[len(carry) // 2 :]
                emit_qk_proj(1)
                for f in carry_a:
                    f()
                emit_qk_proj(2)
                for f in carry_b:
                    f()
                carry = []
                emit_qk_proj(3)
                emit_loads(1)
                emit_v_proj(0)
                emit_v_proj(1)
                flush_transposes(3)
                emit_loads(2)
                emit_v_proj(2)
                emit_v_proj(3)
                emit_qk_proj(4)
                emit_qk_proj(5)
                flush_transposes(5)
                emit_v_proj(4)
                emit_v_proj(5)
                emit_qk_proj(6)
                emit_qk_proj(7)
                flush_transposes(7)
                emit_attn_pair(0, 0, fillers=gen_qk_proj(8))
                emit_v_proj(6)
                flush_transposes(8)
                emit_attn_pair(0, 1, fillers=gen_qk_proj(9))
                emit_v_proj(7)
                emit_loads(3)
                flush_transposes(9)
                emit_load_wo()
                emit_attn_pair(1, 0, fillers=gen_qk_proj(10) + gen_v_proj(8))
                flush_transposes(10)
                emit_attn_pair(1, 1, fillers=gen_qk_proj(11) + gen_v_proj(9))
                flush_transposes(11)
                emit_v_proj(10)
                emit_v_proj(11)
                emit_attn_pair(
                    2, 0,
                    fillers=gen_qk_proj(12) + gen_qk_proj(13)
                    + gen_out_proj(0, 0),
                )
                flush_transposes(13)
                emit_attn_pair(
                    2, 1,
                    fillers=gen_qk_proj(14) + gen_qk_proj(15)
                    + [lambda: flush_transposes(15)] + gen_v_proj(12),
                )
                emit_v_proj(13)
                emit_v_proj(15)
                emit_attn_pair(
                    3, 0,
                    fillers=gen_v_proj(14) + gen_out_proj(0, 1)
                    + gen_out_proj(1, 0) + gen_out_proj(1, 1),
                )
                emit_attn_pair(
                    3, 1, fillers=gen_out_proj(2, 0) + gen_out_proj(2, 1)
                )
                carry = gen_out_proj(3, 0) + gen_out_proj(3, 1)
            for f in carry:
                f()

    nc.compile()
    return nc


def build_in_maps(x, Wq, Wk, Wv, Wo):
    x = np.asarray(x, np.float32)
    Wq = np.asarray(Wq, np.float32)
    Wk = np.asarray(Wk, np.float32)
    Wv = np.asarray(Wv, np.float32)
    Wo = np.asarray(Wo, np.float32)

    # RoPE tables: pos index = sb*128 + partition; pair-frequency index i
    inv = 1.0 / (ROPE_BASE ** (np.arange(0, D, 2, dtype=np.float64) / D))  # [32]
    pos = np.arange(S, dtype=np.float64)
    ang = pos[:, None] * inv[None, :]  # [S, 32]
    cos_t = np.cos(ang).astype(np.float32).reshape(NPB, 128, 32)
    sin_t = np.sin(ang).astype(np.float32).reshape(NPB, 128, 32)
    cosT = np.ascontiguousarray(cos_t.transpose(1, 0, 2)).astype(np.float16)
    sinT = np.concatenate(
        [-sin_t.transpose(1, 0, 2), sin_t.transpose(1, 0, 2)], axis=2
    ).astype(np.float16)  # [128, NPB, 64] = [-sin | +sin]

    # post-exp causal mask for the diagonal 128x128 sub-block: keep (c >= r)
    rr = np.arange(128)[:, None]
    cc = np.arange(128)[None, :]
    maskc = (cc >= rr).astype(np.float16)

    # weight column permutation: even pair-elements then odd (rotate-half)
    perm = np.concatenate([np.arange(0, D, 2), np.arange(1, D, 2)])

    in_maps = []
    for core in range(NCORES):
        b, g = core // HPC, core % HPC
        wqk = np.empty((E, 2 * HPC * D), np.float32)
        for i in range(HPC):
            h = g * HPC + i
            wqk[:, i * D : (i + 1) * D] = Wq[:, h * D : (h + 1) * D][:, perm]
            wqk[:, HPC * D + i * D : HPC * D + (i + 1) * D] = Wk[
                :, h * D : (h + 1) * D
            ][:, perm]
        in_maps.append(
            {
                "xT": np.ascontiguousarray(x[b].T).astype(np.float16),
                "wqk": wqk.astype(np.float16),
                "wv": np.ascontiguousarray(
                    Wv[:, g * HPC * D : (g + 1) * HPC * D]
                ).astype(np.float16),
                "wo": np.ascontiguousarray(
                    Wo[g * HPC * D : (g + 1) * HPC * D, :]
                ).astype(np.float16),
                "cosT": cosT,
                "sinT": sinT,
                "maskc": maskc,
            }
        )
    return in_maps


def gather_output(results):
    outs = [np.asarray(r["out"], np.float32) for r in results]
    return np.stack(
        [outs[0] + outs[1] + outs[2] + outs[3], outs[4] + outs[5] + outs[6] + outs[7]],
        axis=0,
    )


_NC_CACHE = {}


def kernel(x, Wq, Wk, Wv, Wo):
    in_maps = build_in_maps(x, Wq, Wk, Wv, Wo)
    if "nc" not in _NC_CACHE:
        _NC_CACHE["nc"] = build_nc()
    res = run_bass_kernel_spmd(_NC_CACHE["nc"], in_maps, core_ids=list(range(NCORES)))
    return gather_output(res.results)
